# revision 1
# baseline (speedup 1.0000x reference)
"""Trainium2 Bass kernel for the 6-node GCN classification model.

Math: each GCN layer is h' = relu(A @ h @ W^T + b) on [B, 6, 64], where A is
the 6x6 normalized adjacency (with self loops; fill=1.0 for layers 1-2,
fill=2.0 for layers 3-4).  With the 6 nodes stacked in pairs on the 128 SBUF
partitions (2 nodes x 64 features), the fused per-layer operator A (x) W is a
384x384 block matrix; each nonzero 128x128 block becomes one TensorE matmul
accumulated in PSUM.  Node pairings alternate between two stackings chosen so
the total block count over the 4 layers is the provable minimum (26).

Pipeline per 512-batch group, fully fused in SBUF (x is read from HBM exactly
once, only the [B, 6] sigmoid output is written back):
  DMA x (batch-major, contiguous) -> PE transpose to feature-major stacked
  -> 4 x (block matmuls f32r -> ACT bias+ReLU) -> DVE residual add
  -> fc-head matmuls -> ACT sigmoid -> PE transpose back -> DMA out.

Sharding: pure data parallel over the batch dim across the 8 NeuronCores.
"""

import math
from contextlib import ExitStack, nullcontext as _nullctx

import numpy as np

N_CORES = 8
BATCH = 131072
PER_CORE = BATCH // N_CORES  # 16384
NN = 6
FEAT = 64
GROUP = 512
N_GROUPS = PER_CORE // GROUP  # 32

SRC = [1, 2, 0, 2, 1, 3, 2, 4, 3, 5, 3, 4]
DST = [0, 0, 1, 1, 2, 2, 3, 3, 4, 4, 5, 5]

# Node pair stackings per layer boundary (chain start == chain end so the
# residual/fc read the same stacking the input transposes produce).
S_A = [(0, 1), (2, 3), (4, 5)]
S_B = [(0, 5), (1, 2), (3, 4)]
CHAIN = [S_A, S_B, S_A, S_B, S_A]  # layer l maps CHAIN[l] -> CHAIN[l+1]
_IPERMS = [(0, 1, 2), (0, 2, 1), (1, 0, 2), (1, 2, 0), (2, 0, 1), (2, 1, 0)]


def _gcn_A(fill: float) -> np.ndarray:
    """Dense [6, 6] aggregation matrix A[dst, src] incl. weighted self loops."""
    src = SRC + list(range(NN))
    dst = DST + list(range(NN))
    w = [1.0] * len(SRC) + [fill] * NN
    deg = np.zeros(NN, np.float64)
    for s, d, ww in zip(src, dst, w):
        deg[d] += ww
    dinv = np.where(deg > 0, 1.0 / np.sqrt(deg), 0.0)
    A = np.zeros((NN, NN), np.float64)
    for s, d, ww in zip(src, dst, w):
        A[d, s] += dinv[s] * ww * dinv[d]
    return A


def _block_plan():
    """Static plan: for each layer, the nonzero (out_tile, in_tile) blocks.

    Returns [layer][out_tile] -> list of in_tile indices, using the support of
    A (same for both fill values)."""
    S = np.zeros((NN, NN), bool)
    for s, d in zip(SRC, DST):
        S[d, s] = True
    for i in range(NN):
        S[i, i] = True
    plan = []
    for layer in range(4):
        inp, outp = CHAIN[layer], CHAIN[layer + 1]
        lplan = []
        for (n0, n1) in outp:
            js = []
            for j, (m0, m1) in enumerate(inp):
                if S[n0, m0] or S[n0, m1] or S[n1, m0] or S[n1, m1]:
                    js.append(j)
            lplan.append(js)
        plan.append(lplan)
    return plan


BLOCK_PLAN = _block_plan()
N_BLOCKS = sum(len(js) for lp in BLOCK_PLAN for js in lp)  # 26


def build_consts(W, b, fc_w, fc_b):
    """Host-side constant tensors fed to the device as DRAM inputs.

    W: list of 4 [64, 64] arrays; b: list of 4 [64]; fc_w [6, 64]; fc_b [6].
    """
    A = [_gcn_A(1.0), _gcn_A(1.0), _gcn_A(2.0), _gcn_A(2.0)]
    wblk = np.zeros((N_BLOCKS, 128, 128), np.float32)
    k = 0
    for layer in range(4):
        inp, outp = CHAIN[layer], CHAIN[layer + 1]
        Wt = W[layer].T.astype(np.float64)  # [f, g] = W[g, f]
        for i, (n0, n1) in enumerate(outp):
            for j in BLOCK_PLAN[layer][i]:
                m0, m1 = inp[j]
                blk = np.zeros((128, 128), np.float64)
                for dj, m in enumerate((m0, m1)):
                    for do, n in enumerate((n0, n1)):
                        a = A[layer][n, m]
                        if a != 0.0:
                            blk[dj * 64:(dj + 1) * 64, do * 64:(do + 1) * 64] = a * Wt
                wblk[k] = blk.astype(np.float32)
                k += 1
    assert k == N_BLOCKS

    bias = np.zeros((4, 128), np.float32)
    for layer in range(4):
        bias[layer] = np.tile(b[layer], 2)

    fcw = np.zeros((3, 128, NN), np.float32)
    for i, (n0, n1) in enumerate(CHAIN[4]):
        for do, n in enumerate((n0, n1)):
            fcw[i, do * 64:(do + 1) * 64, n] = fc_w[n]

    return {
        "wblk": wblk,
        "bias": bias,
        "fcw": fcw,
        "fcb": fc_b.astype(np.float32).reshape(NN, 1),
        "eye128": np.eye(128, dtype=np.float32),
    }


def build_program(repeats: int = 1, cfg: dict | None = None):
    """Build + schedule + compile the Bass/Tile program. Returns nc."""
    import concourse.tile as tile
    import concourse.mybir as mybir
    from concourse import bacc

    cfg = dict(cfg or {})
    bufs_xb = cfg.get("xb", 3)
    bufs_xs = cfg.get("xs", 3)
    bufs_h = cfg.get("h", 4)
    bufs_r = cfg.get("r", 2)
    bufs_px = cfg.get("px", 2)
    bufs_ph = cfg.get("ph", 5)
    bufs_pfc = cfg.get("pfc", 1)
    bufs_pot = cfg.get("pot", 1)
    xt_in_ph = cfg.get("xt_in_ph", False)
    ot_in_pfc = cfg.get("ot_in_pfc", False)
    wdt_name = cfg.get("wdtype", "f32r")

    f32 = mybir.dt.float32
    f32r = mybir.dt.float32r
    wdt = {"f32r": mybir.dt.float32r, "bf16": mybir.dt.bfloat16}[wdt_name]
    Relu = mybir.ActivationFunctionType.Relu
    Sigmoid = mybir.ActivationFunctionType.Sigmoid

    nc = bacc.Bacc("TRN2", target_bir_lowering=False, debug=False,
                   num_devices=N_CORES)

    x_ap = nc.dram_tensor("x", [PER_CORE, NN * FEAT], f32r,
                          kind="ExternalInput").ap()
    y_ap = nc.dram_tensor("y", [N_GROUPS, NN, GROUP], f32,
                          kind="ExternalOutput").ap()
    wblk_ap = nc.dram_tensor("wblk", [N_BLOCKS, 128, 128], wdt,
                             kind="ExternalInput").ap()
    bias_ap = nc.dram_tensor("bias", [4, 128], f32,
                             kind="ExternalInput").ap()
    fcw_ap = nc.dram_tensor("fcw", [3, 128, NN], wdt,
                            kind="ExternalInput").ap()
    fcb_ap = nc.dram_tensor("fcb", [NN, 1], f32, kind="ExternalInput").ap()
    eye128_ap = nc.dram_tensor("eye128", [128, 128], f32r,
                               kind="ExternalInput").ap()

    SB = GROUP // 128  # 4 batch sub-tiles per group

    with tile.TileContext(nc) as tc, ExitStack() as ctx:
        cpool = ctx.enter_context(tc.tile_pool(name="consts", bufs=1))
        p_xb = ctx.enter_context(tc.tile_pool(name="xb", bufs=bufs_xb))
        p_xs = ctx.enter_context(tc.tile_pool(name="xs", bufs=bufs_xs))
        p_h = ctx.enter_context(tc.tile_pool(name="h", bufs=bufs_h))
        p_r = ctx.enter_context(tc.tile_pool(name="r", bufs=bufs_r))
        p_sig = ctx.enter_context(tc.tile_pool(name="sig", bufs=2))
        p_ob = ctx.enter_context(tc.tile_pool(name="ob", bufs=2))
        p_ph = ctx.enter_context(tc.tile_pool(name="ph", bufs=bufs_ph, space="PSUM"))
        p_px = p_ph if xt_in_ph else ctx.enter_context(
            tc.tile_pool(name="px", bufs=bufs_px, space="PSUM"))
        p_pot = ctx.enter_context(
            tc.tile_pool(name="pot", bufs=bufs_pot, space="PSUM"))
        eye128 = cpool.tile([128, 128], f32r, tag="eye128")
        nc.sync.dma_start(eye128[:], eye128_ap[:])
        btile = cpool.tile([128, 4], f32, tag="bias")
        nc.sync.dma_start(btile[:], bias_ap.rearrange("l p -> p l"))
        bt = [btile[:, layer:layer + 1] for layer in range(4)]
        ftile = cpool.tile([128, 3 * NN], wdt, tag="fcw")
        nc.sync.dma_start(ftile[:].rearrange("p (i n) -> p i n", i=3),
                          fcw_ap.rearrange("i p n -> p i n"))
        fct = [ftile[:, i * NN:(i + 1) * NN] for i in range(3)]
        fcbt = cpool.tile([NN, 1], f32, tag="fcb")
        nc.sync.dma_start(fcbt[:], fcb_ap[:])
        def load_xb(g):
            xb = p_xb.tile([128, SB * NN * FEAT], f32r, tag="xb")
            nc.sync.dma_start(
                xb[:].rearrange("p (s f) -> p s f", s=SB),
                x_ap[g * GROUP:(g + 1) * GROUP, :].rearrange(
                    "(s p) f -> p s f", p=128),
            )
            return xb

        # The first groups' x tiles go before the big weight DMAs so the
        # transposes can start while the weights stream in.  (Single-pass
        # builds only: under a repeat loop these DMAs would not replay.)
        xb_pre = ({g: load_xb(g) for g in range(min(2, N_GROUPS))}
                  if repeats == 1 else {})

        # Block weights in two DMAs (layer-0 blocks first so the first
        # group's matmuls can start before the rest of the weights land).
        nb0 = sum(len(js) for js in BLOCK_PLAN[0])
        wtile = cpool.tile([128, N_BLOCKS * 128], wdt, tag="wblk")
        nc.sync.dma_start(
            wtile[:, :nb0 * 128].rearrange("p (k f) -> p k f", k=nb0),
            wblk_ap[:nb0].rearrange("k p f -> p k f"))
        nc.sync.dma_start(
            wtile[:, nb0 * 128:].rearrange("p (k f) -> p k f", k=N_BLOCKS - nb0),
            wblk_ap[nb0:].rearrange("k p f -> p k f"))
        wt = [wtile[:, k * 128:(k + 1) * 128] for k in range(N_BLOCKS)]


        def group_body(g):
            # Load [512, 384] rows batch-major: partition = batch % 128.
            xb = xb_pre.pop(g, None)
            if xb is None:
                xb = load_xb(g)
            # Transpose to feature-major stacked (pairs = CHAIN[0]).
            first_t = None
            xs = []
            xts = []
            for j in range(3):
                xt = p_px.tile([128, GROUP], f32r,
                               tag="ph" if xt_in_ph else "xt")
                for s in range(SB):
                    ti = nc.tensor.transpose(
                        xt[:, s * 128:(s + 1) * 128],
                        xb[:, s * NN * FEAT + j * 128:
                           s * NN * FEAT + (j + 1) * 128],
                        eye128[:],
                    )
                    if first_t is None:
                        first_t = ti
                xts.append(xt)
            for j in range(3):
                xsj = p_xs.tile([128, GROUP], f32r, tag=f"xs{j}")
                nc.vector.tensor_copy(out=xsj[:], in_=xts[j][:])
                xs.append(xsj)

            h = xs
            iperm = _IPERMS[cfg.get("iorder", 0)]
            woff = [0]
            for layer in range(4):
                for i in range(3):
                    woff.append(woff[-1] + len(BLOCK_PLAN[layer][i]))
            for layer in range(4):
                hn = [None, None, None]
                if cfg.get("pack") and layer in (1, 3):
                    # Layers with in-stacking S_B have two K=64 blocks (only
                    # one node of in-tile 0 feeds them).  Run them as two
                    # concurrent 64x128 row tiles, then the full blocks.
                    ko = woff[layer * 3]
                    ps0 = p_ph.tile([128, GROUP], f32, tag="ph")
                    ps1 = p_ph.tile([128, GROUP], f32, tag="ph")
                    ps2 = p_ph.tile([128, GROUP], f32, tag="ph")
                    kk = lambda i, bi: woff[layer * 3 + i] + bi
                    w_ = lambda k, lo, hi: wtile[lo:hi,
                                                 k * 128:(k + 1) * 128]
                    nc.tensor.matmul(ps0[:], lhsT=w_(kk(0, 0), 0, 64),
                                     rhs=h[0][0:64, :], start=True,
                                     stop=False, tile_position=(0, 0))
                    nc.tensor.matmul(ps2[:], lhsT=w_(kk(2, 0), 64, 128),
                                     rhs=h[0][64:128, :], start=True,
                                     stop=False, tile_position=(64, 0))
                    nc.tensor.matmul(ps0[:], lhsT=wt[kk(0, 1)], rhs=h[1][:],
                                     start=False, stop=True)
                    nc.tensor.matmul(ps1[:], lhsT=wt[kk(1, 0)], rhs=h[1][:],
                                     start=True, stop=False)
                    nc.tensor.matmul(ps1[:], lhsT=wt[kk(1, 1)], rhs=h[2][:],
                                     start=False, stop=True)
                    nc.tensor.matmul(ps2[:], lhsT=wt[kk(2, 1)], rhs=h[2][:],
                                     start=False, stop=True)
                    for i, ps in ((0, ps0), (1, ps1), (2, ps2)):
                        ht = p_h.tile([128, GROUP], f32r, tag=f"h{i}")
                        if layer < 3 and i == 2:
                            nc.vector.tensor_scalar(
                                out=ht[:], in0=ps[:], scalar1=bt[layer],
                                scalar2=0.0, op0=mybir.AluOpType.add,
                                op1=mybir.AluOpType.max)
                        else:
                            nc.scalar.activation(ht[:], ps[:], Relu,
                                                 bias=bt[layer])
                        hn[i] = ht
                    h = hn
                    continue
                for i in iperm:
                    k = woff[layer * 3 + i]
                    ps = p_ph.tile([128, GROUP], f32, tag="ph")
                    js = BLOCK_PLAN[layer][i]
                    for bi, j in enumerate(js):
                        nc.tensor.matmul(
                            ps[:],
                            lhsT=wt[k],
                            rhs=h[j][:],
                            start=(bi == 0),
                            stop=(bi == len(js) - 1),
                        )
                        k += 1
                    ht = p_h.tile([128, GROUP], f32r, tag=f"h{i}")
                    if layer < 3 and i == 2:
                        nc.vector.tensor_scalar(
                            out=ht[:], in0=ps[:], scalar1=bt[layer],
                            scalar2=0.0, op0=mybir.AluOpType.add,
                            op1=mybir.AluOpType.max)
                    else:
                        nc.scalar.activation(ht[:], ps[:], Relu,
                                             bias=bt[layer])
                    hn[i] = ht
                h = hn

            # Residual + fc heads: logits[n, b] accumulate in PSUM [6, 512]
            # with the tiny fc weights stationary (cheap weight loads, full
            # N=512 streams), then sigmoid (+bias) and a strided store
            # straight to the batch-major DRAM layout.
            psfc = p_pot.tile([NN, GROUP], f32, tag="ot")
            first_bm = None
            for i in range(3):
                ri = p_r.tile([128, GROUP], f32r, tag=f"r{i}")
                nc.vector.tensor_add(out=ri[:], in0=h[i][:], in1=xs[i][:])
                mi = nc.tensor.matmul(
                    psfc[:],
                    lhsT=fct[i],
                    rhs=ri[:],
                    start=(i == 0),
                    stop=(i == 2),
                )
                if first_bm is None:
                    first_bm = mi
            sig = p_sig.tile([NN, GROUP], f32, tag="sig")
            nc.scalar.activation(sig[:], psfc[:], Sigmoid, bias=fcbt[:])
            # Store node-major [6, 512] contiguously; the host un-permutes.
            nc.sync.dma_start(y_ap[g], sig[:])
            return first_t, first_bm

        from concourse.tile_rust import add_dep_helper

        def run_groups():
            prev_bm = None
            for g in range(N_GROUPS):
                first_t, first_bm = group_body(g)
                if prev_bm is not None and cfg.get("cluster", False):
                    add_dep_helper(first_t.ins, prev_bm.ins, sync=False,
                                   reason="cluster transpose-mode runs")
                prev_bm = first_bm

        if repeats == 1:
            run_groups()
        else:
            hint = (mybir.EngineType.PE, mybir.EngineType.Activation,
                    mybir.EngineType.DVE, mybir.EngineType.SP,
                    mybir.EngineType.Pool)
            with tc.For_i(0, repeats, hint_engines=hint,
                          staggered_reset=cfg.get("stag", False)):
                run_groups()

    nc.compile()
    return nc


class Runner:
    """Compiled program + cached jitted PJRT executable over the 8 cores.

    Mirrors concourse.bass2jax.run_bass_via_pjrt, but keeps the jitted
    callable and accepts device-resident inputs so repeated timed calls do
    not re-trace or re-transfer."""

    def __init__(self, nc):
        import jax
        import numpy as _np
        from jax.sharding import Mesh, PartitionSpec, NamedSharding
        from jax.experimental.shard_map import shard_map
        import concourse.mybir as mybir
        from concourse import bass2jax

        bass2jax.install_neuronx_cc_hook()
        self.nc = nc
        assert nc.dbg_addr is None
        partition_name = (nc.partition_id_tensor.name
                          if nc.partition_id_tensor else None)

        in_names, out_names, out_avals, zero_outs = [], [], [], []
        for alloc in nc.m.functions[0].allocations:
            if not isinstance(alloc, mybir.MemoryLocationSet):
                continue
            name = alloc.memorylocations[0].name
            if alloc.kind == "ExternalInput":
                if name == partition_name:
                    continue
                in_names.append(name)
            elif alloc.kind == "ExternalOutput":
                shape = tuple(alloc.tensor_shape)
                dtype = mybir.dt.np(alloc.dtype)
                out_names.append(name)
                out_avals.append(jax.core.ShapedArray(shape, dtype))
                zero_outs.append(_np.zeros(shape, dtype))
        self.in_names = list(in_names)
        self.out_names = out_names
        self.out_avals = out_avals
        self.zero_outs = zero_outs
        n_params = len(in_names)
        n_outs = len(out_avals)
        all_in_names = in_names + out_names
        if partition_name is not None:
            all_in_names = all_in_names + [partition_name]
        donate = tuple(range(n_params, n_params + n_outs))

        def _body(*args):
            operands = list(args)
            if partition_name is not None:
                operands.append(bass2jax.partition_id_tensor())
            outs = bass2jax._bass_exec_p.bind(
                *operands,
                out_avals=tuple(out_avals),
                in_names=tuple(all_in_names),
                out_names=tuple(out_names),
                lowering_input_output_aliases=(),
                sim_require_finite=True,
                sim_require_nnan=True,
                nc=nc,
            )
            return tuple(outs)

        devices = jax.devices()[:N_CORES]
        self.mesh = Mesh(_np.asarray(devices), ("core",))
        self.sharding = NamedSharding(self.mesh, PartitionSpec("core"))
        in_specs = (PartitionSpec("core"),) * (n_params + n_outs)
        out_specs = (PartitionSpec("core"),) * n_outs
        self.jitted = jax.jit(
            shard_map(_body, mesh=self.mesh, in_specs=in_specs,
                      out_specs=out_specs, check_rep=False),
            donate_argnums=donate,
            keep_unused=True,
        )
        self._jax = jax

    def put_inputs(self, in_maps):
        """in_maps: list of N_CORES dicts name->np.  Returns device arrays."""
        import numpy as _np
        concat = [
            _np.concatenate([_np.asarray(m[name]) for m in in_maps], axis=0)
            for name in self.in_names
        ]
        return [self._jax.device_put(a, self.sharding) for a in concat]

    def run(self, dev_inputs):
        jax = self._jax
        zeros = [
            jax.device_put(
                self._jax.numpy.zeros((N_CORES * z.shape[0], *z.shape[1:]),
                                      z.dtype),
                self.sharding)
            for z in self.zero_outs
        ]
        outs = self.jitted(*dev_inputs, *zeros)
        outs = [self._jax.block_until_ready(o) for o in outs]
        return {
            name: outs[i]
            for i, name in enumerate(self.out_names)
        }


_RUNNERS = {}


def get_runner(repeats: int = 1, cfg: dict | None = None) -> Runner:
    key = (repeats, tuple(sorted((cfg or {}).items())))
    if key not in _RUNNERS:
        _RUNNERS[key] = Runner(build_program(repeats, cfg))
    return _RUNNERS[key]


def _make_in_maps(inputs, wdtype="f32r"):
    x = np.ascontiguousarray(np.asarray(inputs["x"], np.float32))
    assert x.shape == (BATCH, NN, FEAT)
    consts = build_consts(
        W=[np.asarray(inputs[f"W{i+1}"], np.float32) for i in range(4)],
        b=[np.asarray(inputs[f"b{i+1}"], np.float32) for i in range(4)],
        fc_w=np.asarray(inputs["fc_w"], np.float32),
        fc_b=np.asarray(inputs["fc_b"], np.float32),
    )
    if wdtype == "bf16":
        import ml_dtypes
        consts["wblk"] = consts["wblk"].astype(ml_dtypes.bfloat16)
        consts["fcw"] = consts["fcw"].astype(ml_dtypes.bfloat16)
    x_sh = x.reshape(N_CORES, PER_CORE, NN * FEAT)
    return [{"x": x_sh[c], **consts} for c in range(N_CORES)]


def unpack_y(y_raw: np.ndarray) -> np.ndarray:
    """Device output [N_CORES * N_GROUPS, NN, GROUP] -> [BATCH, NN]."""
    y = y_raw.reshape(N_CORES, N_GROUPS, NN, GROUP)
    return np.ascontiguousarray(
        y.transpose(0, 1, 3, 2).reshape(BATCH, NN))


def kernel(**inputs) -> np.ndarray:
    runner = get_runner(repeats=1)
    dev = runner.put_inputs(_make_in_maps(inputs))
    out = runner.run(dev)
    return unpack_y(np.asarray(out["y"]))



# revision 7
# speedup vs baseline: 1.2280x; 1.2280x over previous
"""Trainium2 Bass kernel for the 6-node GCN classification model.

Math: each GCN layer is h' = relu(A @ h @ W^T + b) on [B, 6, 64], where A is
the 6x6 normalized adjacency (with self loops; fill=1.0 for layers 1-2,
fill=2.0 for layers 3-4).  With the 6 nodes stacked in pairs on the 128 SBUF
partitions (2 nodes x 64 features), the fused per-layer operator A (x) W is a
384x384 block matrix; each nonzero 128x128 block becomes one TensorE matmul
accumulated in PSUM.  Node pairings alternate between two stackings chosen so
the total block count over the 4 layers is the provable minimum (26).

v1 datapath (all bf16):
  - x is converted to bf16 on the host; the HWDGE xbar DMA-transpose loads it
    straight from HBM into feature-major SBUF tiles, so the PE runs ONLY the
    26 block matmuls + 3 fc matmuls per 512-batch group (no PE transposes,
    no PSUM->SBUF input copies).
  - PSUM accumulates in f32; the ReLU+bias PSUM->SBUF(bf16) ops are spread
    across ACT / DVE / Pool so no vector engine exceeds the PE's ~6.2us/group.
  - residual adds run on DVE in bf16 (2x mode), fc heads + sigmoid as before.

Sharding: pure data parallel over the batch dim across the 8 NeuronCores.
"""

from contextlib import ExitStack

import numpy as np

N_CORES = 8
BATCH = 131072
PER_CORE = BATCH // N_CORES  # 16384
NN = 6
FEAT = 64
GROUP = 512
N_GROUPS = PER_CORE // GROUP  # 32

SRC = [1, 2, 0, 2, 1, 3, 2, 4, 3, 5, 3, 4]
DST = [0, 0, 1, 1, 2, 2, 3, 3, 4, 4, 5, 5]

# Node pair stackings per layer boundary (chain start == chain end so the
# residual/fc read the same stacking the input DMA-transposes produce).
S_A = [(0, 1), (2, 3), (4, 5)]
S_B = [(0, 5), (1, 2), (3, 4)]
CHAIN = [S_A, S_B, S_A, S_B, S_A]  # layer l maps CHAIN[l] -> CHAIN[l+1]


def _gcn_A(fill: float) -> np.ndarray:
    """Dense [6, 6] aggregation matrix A[dst, src] incl. weighted self loops."""
    src = SRC + list(range(NN))
    dst = DST + list(range(NN))
    w = [1.0] * len(SRC) + [fill] * NN
    deg = np.zeros(NN, np.float64)
    for s, d, ww in zip(src, dst, w):
        deg[d] += ww
    dinv = np.where(deg > 0, 1.0 / np.sqrt(deg), 0.0)
    A = np.zeros((NN, NN), np.float64)
    for s, d, ww in zip(src, dst, w):
        A[d, s] += dinv[s] * ww * dinv[d]
    return A


def _block_plan():
    """Static plan: for each layer, the nonzero (out_tile, in_tile) blocks."""
    S = np.zeros((NN, NN), bool)
    for s, d in zip(SRC, DST):
        S[d, s] = True
    for i in range(NN):
        S[i, i] = True
    plan = []
    for layer in range(4):
        inp, outp = CHAIN[layer], CHAIN[layer + 1]
        lplan = []
        for (n0, n1) in outp:
            js = []
            for j, (m0, m1) in enumerate(inp):
                if S[n0, m0] or S[n0, m1] or S[n1, m0] or S[n1, m1]:
                    js.append(j)
            lplan.append(js)
        plan.append(lplan)
    return plan


BLOCK_PLAN = _block_plan()
N_BLOCKS = sum(len(js) for lp in BLOCK_PLAN for js in lp)  # 26


def build_consts(W, b, fc_w, fc_b):
    """Host-side constant tensors fed to the device as DRAM inputs."""
    A = [_gcn_A(1.0), _gcn_A(1.0), _gcn_A(2.0), _gcn_A(2.0)]
    wblk = np.zeros((N_BLOCKS, 128, 128), np.float32)
    k = 0
    for layer in range(4):
        inp, outp = CHAIN[layer], CHAIN[layer + 1]
        Wt = W[layer].T.astype(np.float64)  # [f, g] = W[g, f]
        for i, (n0, n1) in enumerate(outp):
            for j in BLOCK_PLAN[layer][i]:
                m0, m1 = inp[j]
                blk = np.zeros((128, 128), np.float64)
                for dj, m in enumerate((m0, m1)):
                    for do, n in enumerate((n0, n1)):
                        a = A[layer][n, m]
                        if a != 0.0:
                            blk[dj * 64:(dj + 1) * 64, do * 64:(do + 1) * 64] = a * Wt
                wblk[k] = blk.astype(np.float32)
                k += 1
    assert k == N_BLOCKS

    bias = np.zeros((4, 128), np.float32)
    for layer in range(4):
        bias[layer] = np.tile(b[layer], 2)

    fcw = np.zeros((3, 128, NN), np.float32)
    for i, (n0, n1) in enumerate(CHAIN[4]):
        for do, n in enumerate((n0, n1)):
            fcw[i, do * 64:(do + 1) * 64, n] = fc_w[n]

    return {
        "wblk": wblk,
        "bias": bias,
        "fcw": fcw,
        "fcb": fc_b.astype(np.float32).reshape(NN, 1),
    }


# ReLU engine per (layer, out_tile): A=ACT, V=DVE (Pool cannot read PSUM).
DEFAULT_RELU_ASGN = "AAV" "AVA" "AVA" "VAV"


def build_program(repeats: int = 1, cfg: dict | None = None):
    """Build + schedule + compile the Bass/Tile program. Returns nc."""
    import concourse.tile as tile
    import concourse.mybir as mybir
    from concourse import bacc

    cfg = dict(cfg or {})
    bufs_xs = cfg.get("xs", 4)
    bufs_h = cfg.get("h", 6)
    bufs_r = cfg.get("r", 3)
    bufs_ph = cfg.get("ph", 6)
    bufs_pfc = cfg.get("pfc", 2)
    relu_asgn = cfg.get("relu", DEFAULT_RELU_ASGN)
    res_eng = cfg.get("res", "VVP")  # residual add engines per tile

    f32 = mybir.dt.float32
    bf16 = mybir.dt.bfloat16
    Relu = mybir.ActivationFunctionType.Relu
    Sigmoid = mybir.ActivationFunctionType.Sigmoid

    nc = bacc.Bacc("TRN2", target_bir_lowering=False, debug=False,
                   num_devices=N_CORES)

    x_ap = nc.dram_tensor("x", [PER_CORE, NN * FEAT], bf16,
                          kind="ExternalInput").ap()
    y_ap = nc.dram_tensor("y", [N_GROUPS, NN, GROUP], f32,
                          kind="ExternalOutput").ap()
    wblk_ap = nc.dram_tensor("wblk", [N_BLOCKS, 128, 128], bf16,
                             kind="ExternalInput").ap()
    bias_ap = nc.dram_tensor("bias", [4, 128], f32,
                             kind="ExternalInput").ap()
    fcw_ap = nc.dram_tensor("fcw", [3, 128, NN], bf16,
                            kind="ExternalInput").ap()
    fcb_ap = nc.dram_tensor("fcb", [NN, 1], f32, kind="ExternalInput").ap()

    with tile.TileContext(nc) as tc, ExitStack() as ctx:
        cpool = ctx.enter_context(tc.tile_pool(name="consts", bufs=1))
        p_xs = ctx.enter_context(tc.tile_pool(name="xs", bufs=bufs_xs))
        p_h = ctx.enter_context(tc.tile_pool(name="h", bufs=bufs_h))
        p_r = ctx.enter_context(tc.tile_pool(name="r", bufs=bufs_r))
        p_sig = ctx.enter_context(tc.tile_pool(name="sig", bufs=2))
        p_ph = ctx.enter_context(tc.tile_pool(name="ph", bufs=bufs_ph, space="PSUM"))
        p_pfc = ctx.enter_context(tc.tile_pool(name="pfc", bufs=bufs_pfc, space="PSUM"))

        btile = cpool.tile([128, 4], f32, tag="bias")
        nc.sync.dma_start(btile[:], bias_ap.rearrange("l p -> p l"))
        bt = [btile[:, layer:layer + 1] for layer in range(4)]
        ftile = cpool.tile([128, 3 * NN], bf16, tag="fcw")
        nc.sync.dma_start(ftile[:].rearrange("p (i n) -> p i n", i=3),
                          fcw_ap.rearrange("i p n -> p i n"))
        fct = [ftile[:, i * NN:(i + 1) * NN] for i in range(3)]
        fcbt = cpool.tile([NN, 1], f32, tag="fcb")
        nc.sync.dma_start(fcbt[:], fcb_ap[:])

        # Block weights in two DMAs (layer-0 blocks first so the first
        # group's matmuls can start before the rest of the weights land).
        nb0 = sum(len(js) for js in BLOCK_PLAN[0])
        wtile = cpool.tile([128, N_BLOCKS * 128], bf16, tag="wblk")
        nc.sync.dma_start(
            wtile[:, :nb0 * 128].rearrange("p (k f) -> p k f", k=nb0),
            wblk_ap[:nb0].rearrange("k p f -> p k f"))
        nc.sync.dma_start(
            wtile[:, nb0 * 128:].rearrange("p (k f) -> p k f", k=N_BLOCKS - nb0),
            wblk_ap[nb0:].rearrange("k p f -> p k f"))
        wt = [wtile[:, k * 128:(k + 1) * 128] for k in range(N_BLOCKS)]

        engines = {"A": nc.scalar, "V": nc.vector, "P": nc.gpsimd}

        def relu_op(eng_key, ht, ps, layer):
            if eng_key == "A":
                nc.scalar.activation(ht[:], ps[:], Relu, bias=bt[layer])
            else:
                engines[eng_key].tensor_scalar(
                    out=ht[:], in0=ps[:], scalar1=bt[layer], scalar2=0.0,
                    op0=mybir.AluOpType.add, op1=mybir.AluOpType.max)

        woff = [0]
        for layer in range(4):
            for i in range(3):
                woff.append(woff[-1] + len(BLOCK_PLAN[layer][i]))
        # Out-tile issue order per layer (fewest-deps first keeps the PE fed).
        tile_order = cfg.get("torder", [[1, 2, 0], [0, 2, 1], [1, 2, 0],
                                        [0, 2, 1]])

        # Stage generators: each stage is a callable emitting that group's
        # instructions for one pipeline step.  Stages of two groups are
        # interleaved so PE never waits on a ReLU of its own group.
        def stage_load(g, st):
            xs = []
            for i in range(3):
                xsi = p_xs.tile([128, GROUP], bf16, tag=f"xs{i}")
                nc.sync.dma_start_transpose(
                    out=xsi[:],
                    in_=x_ap[g * GROUP:(g + 1) * GROUP,
                             i * 128:(i + 1) * 128])
                xs.append(xsi)
            st["xs"] = xs
            st["h"] = xs

        def stage_layer(g, st, layer):
            h = st["h"]
            hn = [None, None, None]
            for i in tile_order[layer]:
                k = woff[layer * 3 + i]
                ps = p_ph.tile([128, GROUP], f32, tag="ph")
                js = BLOCK_PLAN[layer][i]
                for bi, j in enumerate(js):
                    nc.tensor.matmul(
                        ps[:],
                        lhsT=wt[k + bi],
                        rhs=h[j][:],
                        start=(bi == 0),
                        stop=(bi == len(js) - 1),
                    )
                ht = p_h.tile([128, GROUP], bf16, tag=f"h{i}")
                relu_op(relu_asgn[layer * 3 + i], ht, ps, layer)
                hn[i] = ht
            st["h"] = hn

        def stage_out(g, st):
            h, xs = st["h"], st["xs"]
            psfc = p_pfc.tile([NN, GROUP], f32, tag="fc")
            for i in range(3):
                ri = p_r.tile([128, GROUP], bf16, tag=f"r{i}")
                eng = engines[res_eng[i]]
                eng.tensor_tensor(out=ri[:], in0=h[i][:], in1=xs[i][:],
                                  op=mybir.AluOpType.add)
                nc.tensor.matmul(
                    psfc[:],
                    lhsT=fct[i],
                    rhs=ri[:],
                    start=(i == 0),
                    stop=(i == 2),
                )
            sig = p_sig.tile([NN, GROUP], f32, tag="sig")
            nc.scalar.activation(sig[:], psfc[:], Sigmoid, bias=fcbt[:])
            # Store node-major [6, 512] contiguously; the host un-permutes.
            nc.sync.dma_start(y_ap[g], sig[:])

        STAGES = [stage_load,
                  lambda g, st: stage_layer(g, st, 0),
                  lambda g, st: stage_layer(g, st, 1),
                  lambda g, st: stage_layer(g, st, 2),
                  lambda g, st: stage_layer(g, st, 3),
                  stage_out]

        ilv = cfg.get("ilv", 2)

        def run_groups():
            # Software-pipeline `ilv` groups: emit stage s of all groups in
            # the cohort before stage s+1 of any.
            for g0 in range(0, N_GROUPS, ilv):
                gs = list(range(g0, min(g0 + ilv, N_GROUPS)))
                sts = {g: {} for g in gs}
                for stage in STAGES:
                    for g in gs:
                        stage(g, sts[g])

        if repeats == 1:
            run_groups()
        else:
            hint = (mybir.EngineType.PE, mybir.EngineType.Activation,
                    mybir.EngineType.DVE, mybir.EngineType.SP,
                    mybir.EngineType.Pool)
            with tc.For_i(0, repeats, hint_engines=hint):
                run_groups()

    nc.compile()
    return nc


class Runner:
    """Compiled program + cached jitted PJRT executable over the 8 cores."""

    def __init__(self, nc):
        import jax
        import numpy as _np
        from jax.sharding import Mesh, PartitionSpec, NamedSharding
        from jax.experimental.shard_map import shard_map
        import concourse.mybir as mybir
        from concourse import bass2jax

        bass2jax.install_neuronx_cc_hook()
        self.nc = nc
        assert nc.dbg_addr is None
        partition_name = (nc.partition_id_tensor.name
                          if nc.partition_id_tensor else None)

        in_names, out_names, out_avals, zero_outs = [], [], [], []
        for alloc in nc.m.functions[0].allocations:
            if not isinstance(alloc, mybir.MemoryLocationSet):
                continue
            name = alloc.memorylocations[0].name
            if alloc.kind == "ExternalInput":
                if name == partition_name:
                    continue
                in_names.append(name)
            elif alloc.kind == "ExternalOutput":
                shape = tuple(alloc.tensor_shape)
                dtype = mybir.dt.np(alloc.dtype)
                out_names.append(name)
                out_avals.append(jax.core.ShapedArray(shape, dtype))
                zero_outs.append(_np.zeros(shape, dtype))
        self.in_names = list(in_names)
        self.out_names = out_names
        self.out_avals = out_avals
        self.zero_outs = zero_outs
        n_params = len(in_names)
        n_outs = len(out_avals)
        all_in_names = in_names + out_names
        if partition_name is not None:
            all_in_names = all_in_names + [partition_name]
        donate = tuple(range(n_params, n_params + n_outs))

        def _body(*args):
            operands = list(args)
            if partition_name is not None:
                operands.append(bass2jax.partition_id_tensor())
            outs = bass2jax._bass_exec_p.bind(
                *operands,
                out_avals=tuple(out_avals),
                in_names=tuple(all_in_names),
                out_names=tuple(out_names),
                lowering_input_output_aliases=(),
                sim_require_finite=True,
                sim_require_nnan=True,
                nc=nc,
            )
            return tuple(outs)

        devices = jax.devices()[:N_CORES]
        self.mesh = Mesh(_np.asarray(devices), ("core",))
        self.sharding = NamedSharding(self.mesh, PartitionSpec("core"))
        in_specs = (PartitionSpec("core"),) * (n_params + n_outs)
        out_specs = (PartitionSpec("core"),) * n_outs
        self.jitted = jax.jit(
            shard_map(_body, mesh=self.mesh, in_specs=in_specs,
                      out_specs=out_specs, check_rep=False),
            donate_argnums=donate,
            keep_unused=True,
        )
        self._jax = jax

    def put_inputs(self, in_maps):
        """in_maps: list of N_CORES dicts name->np.  Returns device arrays."""
        import numpy as _np
        concat = [
            _np.concatenate([_np.asarray(m[name]) for m in in_maps], axis=0)
            for name in self.in_names
        ]
        return [self._jax.device_put(a, self.sharding) for a in concat]

    def run(self, dev_inputs):
        jax = self._jax
        zeros = [
            jax.device_put(
                self._jax.numpy.zeros((N_CORES * z.shape[0], *z.shape[1:]),
                                      z.dtype),
                self.sharding)
            for z in self.zero_outs
        ]
        outs = self.jitted(*dev_inputs, *zeros)
        outs = [self._jax.block_until_ready(o) for o in outs]
        return {
            name: outs[i]
            for i, name in enumerate(self.out_names)
        }


_RUNNERS = {}


def get_runner(repeats: int = 1, cfg: dict | None = None) -> Runner:
    key = (repeats, tuple(sorted((cfg or {}).items())))
    if key not in _RUNNERS:
        _RUNNERS[key] = Runner(build_program(repeats, cfg))
    return _RUNNERS[key]


def _make_in_maps(inputs):
    import ml_dtypes
    x = np.ascontiguousarray(np.asarray(inputs["x"], np.float32))
    assert x.shape == (BATCH, NN, FEAT)
    consts = build_consts(
        W=[np.asarray(inputs[f"W{i+1}"], np.float32) for i in range(4)],
        b=[np.asarray(inputs[f"b{i+1}"], np.float32) for i in range(4)],
        fc_w=np.asarray(inputs["fc_w"], np.float32),
        fc_b=np.asarray(inputs["fc_b"], np.float32),
    )
    consts["wblk"] = consts["wblk"].astype(ml_dtypes.bfloat16)
    consts["fcw"] = consts["fcw"].astype(ml_dtypes.bfloat16)
    x_sh = x.reshape(N_CORES, PER_CORE, NN * FEAT).astype(ml_dtypes.bfloat16)
    return [{"x": x_sh[c], **consts} for c in range(N_CORES)]


def unpack_y(y_raw: np.ndarray) -> np.ndarray:
    """Device output [N_CORES * N_GROUPS, NN, GROUP] -> [BATCH, NN]."""
    y = y_raw.reshape(N_CORES, N_GROUPS, NN, GROUP)
    return np.ascontiguousarray(
        y.transpose(0, 1, 3, 2).reshape(BATCH, NN))


def kernel(**inputs) -> np.ndarray:
    runner = get_runner(repeats=1)
    dev = runner.put_inputs(_make_in_maps(inputs))
    out = runner.run(dev)
    return unpack_y(np.asarray(out["y"]))
